# revision 29
# baseline (speedup 1.0000x reference)
"""AttentionRNNCell Trainium2 kernel (v2: fp8 dual-layout, no on-chip transposes).

Math (per batch row b):
  et[t]  = V_a . tanh( (h W_a + b_a) + x[t] U_a )        t in [0, TE)
  at     = exp(et);  s = sum(at)
  ctx    = (sum_t at[t] x[t]) / s
  zt     = sigmoid(h W_z + [inp, ctx] C_z + b_z)
  rt     = sigmoid(h W_r + [inp, ctx] C_r + b_r)
  tht    = tanh((rt*h) U_p + [inp, ctx] C_p + b_p)
  ht     = (1-zt)*h + zt*tht

Distribution: data-parallel over batch B=128 across 8 cores (16 rows each).
Per core, the 16 rows are processed in 4 groups of 4 rows; within a group the
4 rows map to the 4 PE column strips (tile_position=(0,32s)) so their N=1-ish
matmuls run concurrently.

Key structure (per core):
  - host sends x twice in fp8e4m3: xt8 (transposed, DoubleRow-interleaved
    [b, e%128, e//128, t]) and xn8 (natural [b, t%128, t//128, e]); 16 MB HBM
    instead of 33.5 MB fp32 + 633 us of on-chip DMA transposes.
  - uxpb = U_a^T x^T via fp8 DoubleRow matmul (contraction 256 in one pass,
    U_a stationary) -> PSUM [u%128, t-chunk]
  - ACT tanh (bias = per-partition (h W_a + b_a) column) -> SBUF fp8
  - et: V_a zero-padded to [128, 32] stationary per column strip; 4 rows'
    tanh stream concurrently; junk partitions get exact 0
  - ACT exp over [128, 1024] psum chunks -> at (bf16)
  - at^T via regular matmul with a [128, 4] selection matrix as moving
    operand (out = at^T @ sel); DVE scatter-cast to zero-padded fp8
    stationaries; context matmuls col-tiled over strips, rhs = xn8
    (whose extra ones-column also yields the at row sums)
  - context normalized in the strip-partition domain (recip broadcast along
    free), then transposed to [e, b] via sel-matmul for the gate tail
  - gates: bf16 weights, transposed [u, b] orientation; sigmoid via tanh
    identity (single ACT table set for the whole kernel)
  - emission is software-pipelined at half-group granularity: each stage's
    uxpb+tanh is emitted before the previous stage's attention phase, so the
    scheduler (priority = emission order) keeps ACT -- the bottleneck at
    ~97% busy -- continuously fed while attention matmuls fill PE slack
"""

from contextlib import ExitStack

import numpy as np
import ml_dtypes

import concourse.bass as bass
import concourse.mybir as mybir
import concourse.tile as tile

BF16 = ml_dtypes.bfloat16
E4 = ml_dtypes.float8_e4m3
F32 = mybir.dt.float32
BF = mybir.dt.bfloat16
F8 = mybir.dt.float8e4
AF = mybir.ActivationFunctionType
DR = mybir.MatmulPerfMode.DoubleRow

B, TE, U, IN_DIM = 128, 2048, 256, 256
N_CORES = 8
BS = B // N_CORES  # 16 batch rows per core
P = 128
NS = 4             # column strips (rows per group)
UC = U // P        # 2
UE = U + 8         # xn8 e-width: col U = 1.0 (at row-sum via ctx matmul)


def split_multi_waits(nc, max_waits=1):
    """This container's walrus rejects instructions carrying more than one
    sync wait. Hoist extra waits onto standalone same-engine NoOps inserted
    immediately before the offending instruction."""
    n_new = 0
    for f in nc.m.functions:
        for blk in f.blocks:
            new_insts = []
            for inst in blk.instructions:
                si = inst.sync_info
                waits = list(si.on_wait) if si and si.on_wait else []
                if len(waits) > max_waits:
                    for w in waits[:-max_waits]:
                        nop = mybir.InstNoOp(
                            name=f"{inst.name}-hw{n_new}", ins=[], outs=[]
                        )
                        nop.engine = inst.engine
                        nop.sync_info = mybir.SyncInfo(on_wait=[w], on_update=[])
                        new_insts.append(nop)
                        n_new += 1
                    si.on_wait = waits[-max_waits:]
                new_insts.append(inst)
            blk.instructions = new_insts
    return n_new


def build_nc(bs=BS, te=TE, split_waits=True):
    ng = bs // NS          # row groups (4)
    t512 = te // 512       # 512-wide t chunks (4)
    tc_n = te // P         # 128-wide t chunks (16)

    nc = bass.Bass()
    xt8_d = nc.declare_dram_parameter("xt8", [bs, P, 2, te], F8, isOutput=False)
    xn8_d = nc.declare_dram_parameter("xn8", [bs, P, tc_n, UE], F8, isOutput=False)
    ua8_d = nc.declare_dram_parameter("ua8", [P, 2, UC, P], F8, isOutput=False)
    vpad_d = nc.declare_dram_parameter("vpad", [P, UC, 32], F8, isOutput=False)
    wxpbT_d = nc.declare_dram_parameter("wxpbT", [P, UC, bs], F32, isOutput=False)
    selb_d = nc.declare_dram_parameter("selb", [P, NS], BF, isOutput=False)
    self32_d = nc.declare_dram_parameter("self32", [P, NS], F32, isOutput=False)
    hT_d = nc.declare_dram_parameter("hT", [U, bs], F32, isOutput=False)
    g0T_d = nc.declare_dram_parameter("g0T", [3, U, bs], F32, isOutput=False)
    cz_d = nc.declare_dram_parameter("cz", [U, U], BF, isOutput=False)
    cr_d = nc.declare_dram_parameter("cr", [U, U], BF, isOutput=False)
    cp_d = nc.declare_dram_parameter("cp", [U, U], BF, isOutput=False)
    up_d = nc.declare_dram_parameter("up", [U, U], BF, isOutput=False)
    id_d = nc.declare_dram_parameter("ident", [P, P], F32, isOutput=False)
    ht_d = nc.declare_dram_parameter("ht", [bs, U], F32, isOutput=True)

    with tile.TileContext(nc) as tc, ExitStack() as ctx:
        singles = ctx.enter_context(tc.tile_pool(name="singles", bufs=1))
        xt_p = ctx.enter_context(tc.tile_pool(name="xt", bufs=3))
        xn_p = ctx.enter_context(tc.tile_pool(name="xn", bufs=3))
        th_p = ctx.enter_context(tc.tile_pool(name="th", bufs=3))
        at_p = ctx.enter_context(tc.tile_pool(name="at", bufs=2))
        sm_p = ctx.enter_context(tc.tile_pool(name="sm", bufs=2))
        ux_ps = ctx.enter_context(tc.tile_pool(name="uxps", bufs=2, space="PSUM"))
        et_ps = ctx.enter_context(tc.tile_pool(name="etps", bufs=1, space="PSUM"))
        atT_ps = ctx.enter_context(tc.tile_pool(name="atTps", bufs=1, space="PSUM"))
        ctx_ps = ctx.enter_context(tc.tile_pool(name="ctxps", bufs=1, space="PSUM"))

        # ---- setup: only what the first uxpb/tanh needs, on the fast
        # HWDGE ring; everything else is emitted after the first front
        # stage so it can't delay the pipeline ramp.
        ua8 = singles.tile([P, 2, UC, P], F8)
        nc.sync.dma_start(out=ua8, in_=ua8_d[:, :, :, :])
        wxpb = singles.tile([P, UC, bs], F32)
        nc.sync.dma_start(out=wxpb, in_=wxpbT_d[:, :, :])
        vpad = singles.tile([P, UC, 32], F8)
        nc.sync.dma_start(out=vpad, in_=vpad_d[:, :, :])

        def emit_setup_rest():
            nonlocal selb, self32, hT_sb, g0_sb, id_sb
            # placeholder
            selb = singles.tile([P, NS], BF)
            nc.sync.dma_start(out=selb, in_=selb_d[:, :])
            self32 = singles.tile([P, NS], F32)
            nc.sync.dma_start(out=self32, in_=self32_d[:, :])
            hT_sb = singles.tile([P, UC, bs], F32)
            nc.sync.dma_start(out=hT_sb, in_=hT_d[:, :].rearrange("(c p) b -> p c b", p=P))
            g0_sb = singles.tile([P, 3, UC, bs], F32)
            nc.sync.dma_start(out=g0_sb, in_=g0T_d[:, :, :].rearrange("g (c p) b -> p g c b", p=P))
            for name, d in (("cz", cz_d), ("cr", cr_d), ("cp", cp_d), ("up", up_d)):
                w_sb = singles.tile([P, UC, U], BF, name=f"{name}_sb")
                nc.sync.dma_start(out=w_sb, in_=d[:, :].rearrange("(c p) u -> p c u", p=P))
                gate_w[name] = w_sb
            id_sb = singles.tile([P, P], F32)
            nc.sync.dma_start(out=id_sb, in_=id_d[:, :])
            for i in range(2):
                t8 = singles.tile([P, tc_n, NS, 32], F8, name=f"atT8_{i}")
                nc.vector.memset(t8, 0.0)
                atT8_tiles.append(t8)

        selb = self32 = hT_sb = g0_sb = id_sb = None
        gate_w = {}
        ctxT_all = singles.tile([P, UC, bs], BF)
        atT8_tiles = []

        # ---- streaming loop: software-pipelined at half-group granularity.
        # Each stage (g, h) covers rows 4g..4g+3, t-range [h*te/2, (h+1)*te/2).
        # We emit stage K's uxpb+tanh BEFORE stage K-1's attention phase so
        # the scheduler (priority = emission order) always prefers feeding
        # ACT with tanh work; V-dot/at/ctx fill PE slack.
        th = te // 2
        ux_n = 1024 if th % 1024 == 0 else min(512, th)  # uxpb psum tile width
        mm_n = min(512, ux_n)  # matmul N per psum bank
        grp = {}  # g -> group-lifetime tiles

        def emit_front(g, h):
            if h == 0:
                grp[g] = {
                    "xt8": xt_p.tile([P, NS, 2, te], F8, tag="xt8", name=f"xt{g}"),
                    "xn8": xn_p.tile([P, NS, tc_n, UE], F8, tag="xn8", name=f"xn{g}"),
                    "th_sb": th_p.tile([P, NS, UC, te], F8, tag="th", name=f"th{g}"),
                }
            st = grp[g]
            xt8, xn8, th_sb = st["xt8"], st["xn8"], st["th_sb"]
            for s in range(NS):
                nc.gpsimd.dma_start(
                    out=xt8[:, s, :, h * th : h * th + th],
                    in_=xt8_d[NS * g + s, :, :, h * th : h * th + th],
                )
            for s in range(NS):
                nc.gpsimd.dma_start(
                    out=xn8[:, s, h * (tc_n // 2) : (h + 1) * (tc_n // 2)],
                    in_=xn8_d[NS * g + s, :, h * (tc_n // 2) : (h + 1) * (tc_n // 2)],
                )
            for uc in range(UC):
                for th2 in range(th // ux_n):
                    o = h * th + th2 * ux_n
                    for s in range(NS):
                        ux = ux_ps.tile([P, ux_n], F32, tag="ux", name=f"ux{uc}{s}{th2}")
                        for h2 in range(ux_n // mm_n):
                            nc.tensor.matmul(
                                out=ux[:, h2 * mm_n : h2 * mm_n + mm_n],
                                lhsT=ua8[:, :, uc, :],
                                rhs=xt8[:, s, :, o + h2 * mm_n : o + h2 * mm_n + mm_n],
                                perf_mode=DR, start=True, stop=True,
                            )
                        nc.scalar.activation(
                            out=th_sb[:, s, uc, o : o + ux_n], in_=ux,
                            func=AF.Tanh, bias=wxpb[:, uc, NS * g + s : NS * g + s + 1],
                        )

        def emit_et(g, h):
            # V-dot + exp: emitted BEFORE the next front stage so the exps
            # slot between tanh batches in the ACT stream (one-stage lag)
            st = grp[g]
            th_sb = st["th_sb"]
            if h == 0:
                st["at_sb"] = at_p.tile([P, te], BF, tag="at", name=f"at{g}")
                st["cps"] = ctx_ps.tile([P, 512], F32, tag="ctx", name=f"cps{g}")
            at_sb = st["at_sb"]

            # et: V-padded stationaries, 4 strips concurrent; junk rows = 0
            et_n = ux_n
            for t10 in range(th // et_n):
                o = h * th + t10 * et_n
                et = et_ps.tile([P, et_n], F32, tag="et", name=f"et{t10}")
                for h5 in range(et_n // mm_n):
                    for uc in range(UC):
                        for s in range(NS):
                            nc.tensor.matmul(
                                out=et[32 * s : 32 * s + 32, h5 * mm_n : h5 * mm_n + mm_n],
                                lhsT=vpad[:, uc, :],
                                rhs=th_sb[:, s, uc, o + h5 * mm_n : o + h5 * mm_n + mm_n],
                                start=(uc == 0), stop=(uc == UC - 1),
                                tile_position=(0, 32 * s), skip_group_check=True,
                            )
                nc.scalar.activation(out=at_sb[:, o : o + et_n], in_=et, func=AF.Exp)

        def emit_back(g, h):
            st = grp[g]
            xn8 = st["xn8"]
            at_sb, cps = st["at_sb"], st["cps"]

            # at^T via sel-matmul; scatter-cast into zero-padded fp8 stationaries
            hj = tc_n // 2
            atT = atT_ps.tile([P, hj, NS], F32, tag="atT", name=f"atT{g}{h}")
            for j in range(hj):
                nc.tensor.matmul(
                    out=atT[:, j, :],
                    lhsT=at_sb[:, (h * hj + j) * P : (h * hj + j + 1) * P],
                    rhs=selb, start=True, stop=True,
                )
            atT8 = atT8_tiles[g % 2]
            nc.vector.tensor_copy(atT8[:, h * hj : h * hj + hj, :, 0:1], atT[:, :, :, None])

            # context (+ at row-sum via the ones column of xn8):
            # col-tiled at-stationary matmuls, rhs = xn8
            for j in range(h * hj, h * hj + hj):
                for s in range(NS):
                    nc.tensor.matmul(
                        out=cps[32 * s : 32 * s + 32, :UE],
                        lhsT=atT8[:, j, s, :],
                        rhs=xn8[:, s, j, :],
                        start=(j == 0), stop=(j == tc_n - 1),
                        tile_position=(0, 32 * s), skip_group_check=True,
                    )
            if h == 0:
                return

            # normalize + ctx^T -> [e, b] columns for the gate tail
            recip = sm_p.tile([P, 1], F32, tag="recip", name="recip")
            nc.vector.tensor_scalar_max(recip, cps[:, U : U + 1], 1e-30)
            nc.vector.reciprocal(recip, recip)
            ctxn = sm_p.tile([P, U], F32, tag="ctxn", name="ctxn")
            nc.vector.tensor_mul(ctxn, cps[:, :U], recip.broadcast_to([P, U]))
            ctT = atT_ps.tile([P, UC, NS], F32, tag="atT", name=f"ctT{g}")
            for ec in range(UC):
                nc.tensor.matmul(
                    out=ctT[:, ec, :], lhsT=ctxn[:, ec * P : (ec + 1) * P],
                    rhs=self32, start=True, stop=True,
                )
            nc.vector.tensor_copy(ctxT_all[:, :, NS * g : NS * g + NS], ctT)

        emit_setup_rest()
        prev = None
        for g in range(ng):
            for h in range(2):
                emit_front(g, h)
                if prev is not None:
                    emit_et(*prev)
                    emit_back(*prev)
                prev = (g, h)
        emit_et(*prev)
        emit_back(*prev)

        # ---- tail: gates, output (fp32, transposed [u, b] orientation) ----
        def gate_psum(w_names_rhs, name):
            outs = []
            for uc in range(UC):
                gt = et_ps.tile([P, bs], F32, tag="et", name=f"{name}{uc}")
                n_mm = sum(UC for _ in w_names_rhs)
                i = 0
                for w_sb, rhs_fn in w_names_rhs:
                    for e in range(UC):
                        nc.tensor.matmul(
                            out=gt,
                            lhsT=w_sb[:, e, uc * P : (uc + 1) * P],
                            rhs=rhs_fn(e),
                            start=(i == 0), stop=(i == n_mm - 1),
                        )
                        i += 1
                outs.append(gt)
            return outs

        # zt^T, rt^T = sigmoid(g0 + C_*ctx^T ctx^T); sigmoid(x)=.5*tanh(.5x)+.5
        zt_sb = sm_p.tile([P, UC, bs], F32, tag="zt", name="zt")
        rt_sb = sm_p.tile([P, UC, bs], F32, tag="rt", name="rt")
        for gi, (wname, dst) in enumerate((("cz", zt_sb), ("cr", rt_sb))):
            gps = gate_psum([(gate_w[wname], lambda e: ctxT_all[:, e, :])], wname)
            for uc in range(UC):
                tmp = sm_p.tile([P, bs], F32, tag="gtmp", name=f"t{wname}{uc}")
                nc.vector.tensor_add(tmp, gps[uc], g0_sb[:, gi, uc, :])
                tmp2 = sm_p.tile([P, bs], F32, tag="gtmp", name=f"u{wname}{uc}")
                nc.scalar.activation(out=tmp2, in_=tmp, func=AF.Tanh, scale=0.5)
                nc.vector.tensor_scalar_mul(tmp2, tmp2, 0.5)
                nc.vector.tensor_scalar_add(dst[:, uc, :], tmp2, 0.5)

        # rh^T = rt^T * h^T ; tht^T = tanh(g0p + U_p^T rh^T + C_p^T ctx^T)
        rh_sb = sm_p.tile([P, UC, bs], BF, tag="rh", name="rh")
        for uc in range(UC):
            nc.vector.tensor_mul(rh_sb[:, uc, :], rt_sb[:, uc, :], hT_sb[:, uc, :])
        gps = gate_psum(
            [(gate_w["up"], lambda e: rh_sb[:, e, :]),
             (gate_w["cp"], lambda e: ctxT_all[:, e, :])],
            "cp",
        )
        ht_nat = sm_p.tile([bs, U], F32, tag="htnat", name="ht_nat")
        for uc in range(UC):
            tmp = sm_p.tile([P, bs], F32, tag="gtmp", name=f"tp{uc}")
            nc.vector.tensor_add(tmp, gps[uc], g0_sb[:, 2, uc, :])
            tht = sm_p.tile([P, bs], F32, tag="gtmp", name=f"tht{uc}")
            nc.scalar.activation(out=tht, in_=tmp, func=AF.Tanh)
            # ht^T = h^T + zt^T*(tht^T - h^T)
            nc.vector.tensor_sub(tht, tht, hT_sb[:, uc, :])
            nc.vector.tensor_mul(tht, tht, zt_sb[:, uc, :])
            nc.vector.tensor_add(tht, tht, hT_sb[:, uc, :])
            tp = et_ps.tile([bs, P], F32, tag="et", name=f"htp{uc}")
            nc.tensor.matmul(out=tp, lhsT=tht, rhs=id_sb, start=True, stop=True)
            nc.vector.tensor_copy(ht_nat[:, uc * P : (uc + 1) * P], tp)
        nc.sync.dma_start(out=ht_d[:, :], in_=ht_nat)

    if split_waits:
        split_multi_waits(nc)
    return nc


def _host_prep(inputs, h_tm, V_a, W_a, U_a, b_a, C_z, W_z, b_z, C_r, W_r, b_r,
               C_p, U_p, b_p, bs=BS, n_cores=N_CORES):
    """Fold everything not depending on x_seq into small per-core tensors."""
    wxpb = h_tm @ W_a + b_a                                # [B, U]
    g_z0 = h_tm @ W_z + inputs @ C_z[:IN_DIM] + b_z        # [B, U]
    g_r0 = h_tm @ W_r + inputs @ C_r[:IN_DIM] + b_r
    g_p0 = inputs @ C_p[:IN_DIM] + b_p

    ua8 = np.ascontiguousarray(
        U_a.reshape(2, P, UC, P).transpose(1, 0, 2, 3).astype(E4)
    )
    vpad = np.zeros((P, UC, 32), dtype=E4)
    for uc in range(UC):
        vpad[:, uc, 0] = V_a[P * uc : P * (uc + 1)].astype(E4)
    selb = np.zeros((P, NS), dtype=BF16)
    self32 = np.zeros((P, NS), dtype=np.float32)
    for s in range(NS):
        selb[32 * s, s] = 1.0
        self32[32 * s, s] = 1.0
    shared = {
        "ua8": ua8, "vpad": vpad, "selb": selb, "self32": self32,
        "cz": np.ascontiguousarray(C_z[IN_DIM:].astype(BF16)),
        "cr": np.ascontiguousarray(C_r[IN_DIM:].astype(BF16)),
        "cp": np.ascontiguousarray(C_p[IN_DIM:].astype(BF16)),
        "up": np.ascontiguousarray(U_p.astype(BF16)),
        "ident": np.eye(P, dtype=np.float32),
    }
    per_core = []
    for c in range(n_cores):
        sl = slice(c * bs, (c + 1) * bs)
        per_core.append(
            {
                "wxpbT": np.ascontiguousarray(
                    wxpb[sl].T.reshape(UC, P, bs).transpose(1, 0, 2).astype(np.float32)
                ),
                "hT": np.ascontiguousarray(h_tm[sl].T.astype(np.float32)),
                "g0T": np.ascontiguousarray(
                    np.stack([g_z0[sl].T, g_r0[sl].T, g_p0[sl].T]).astype(np.float32)
                ),
                **shared,
            }
        )
    return per_core


def _x_layouts(x_core, te=TE):
    """Per-core x -> (xt8, xn8) fp8 device layouts."""
    bs = x_core.shape[0]
    tc_n = te // P
    # (b, p, i, t) = x[b, t, 128i+p]
    xt8 = np.ascontiguousarray(
        x_core.transpose(0, 2, 1).reshape(bs, 2, P, te).transpose(0, 2, 1, 3)
        .astype(E4)
    )
    # (b, p, tc, e) = x[b, 128*tc+p, e]; col U = 1.0 (at row-sum), rest 0
    xn = np.zeros((bs, tc_n, P, UE), dtype=E4)
    xn[:, :, :, :U] = x_core.reshape(bs, tc_n, P, U).astype(E4)
    xn[:, :, :, U] = 1.0
    xn8 = np.ascontiguousarray(xn.transpose(0, 2, 1, 3))
    return xt8, xn8


def kernel(inputs, h_tm, x_seq, V_a, W_a, U_a, b_a, C_z, W_z, b_z,
           C_r, W_r, b_r, C_p, U_p, b_p):
    from concourse.bass_utils import run_bass_kernel_spmd

    args = {k: np.asarray(v, dtype=np.float32) for k, v in dict(
        inputs=inputs, h_tm=h_tm, V_a=V_a, W_a=W_a, U_a=U_a, b_a=b_a,
        C_z=C_z, W_z=W_z, b_z=b_z, C_r=C_r, W_r=W_r, b_r=b_r,
        C_p=C_p, U_p=U_p, b_p=b_p).items()}
    x_seq = np.asarray(x_seq, dtype=np.float32)

    per_core = _host_prep(**args)
    in_maps = []
    for c in range(N_CORES):
        m = dict(per_core[c])
        m["xt8"], m["xn8"] = _x_layouts(x_seq[c * BS : (c + 1) * BS])
        in_maps.append(m)

    nc = build_nc()
    res = run_bass_kernel_spmd(nc, in_maps, core_ids=list(range(N_CORES)))
    return np.concatenate([res.results[c]["ht"] for c in range(N_CORES)], axis=0)
